# revision 1
# baseline (speedup 1.0000x reference)
"""CrossCorrAngleEstimator Trainium2 kernel (V2: overlap-save DFT on TensorE).

Full inputs: audio [2048, 2, 4800] f32 -> (optimal_lag i32[2048], theta f32, conf f32).
Data-parallel over batch: 8 cores x 256. Per core:
  corr[b, L] = sum_n a0[b, n+L] a1[b, n], L in [-20, 20], via overlap-save:
  23 chunks of C=214 samples, M=254-point real DFT (F=128 freqs) done as
  float32r matmuls with fixed DFT matrices as stationary weights; cross-spectra
  multiplied on DVE; inverse DFT (41 lags) accumulates all chunks in PSUM.
  Tail: abs -> top-8 max/argmax -> theta LUT -> confidence tanh.
"""

import sys

sys.path.insert(0, "/opt/trn_rl_repo")

import numpy as np

BATCH = 2048
WIN = 4800
MAX_DELAY = 20
NLAGS = 41
N_CORES = 8
B_CORE = 256
FS = 48000
D_MIC = 0.1
C_SOUND = 343.0

C = 214  # chunk hop
M = 254  # DFT size (C + 2*MAX_DELAY)
F = M // 2 + 1  # 128 real freqs
K = 23  # ceil(4800 / C)

_cache = {}


def _theta_table():
    lags = np.arange(-MAX_DELAY, MAX_DELAY + 1).astype(np.float32)
    k = np.float32(C_SOUND / (D_MIC * FS))
    st = np.clip(lags * k, np.float32(-1), np.float32(1)).astype(np.float32)
    return (
        np.float32(90.0) - np.arcsin(st).astype(np.float32) * np.float32(180.0 / np.pi)
    ).astype(np.float32)


def _build():
    import concourse.bacc as bacc
    import concourse.mybir as mybir
    from concourse import tile

    f32 = mybir.dt.float32
    f32r = mybir.dt.float32r
    Alu = mybir.AluOpType
    Act = mybir.ActivationFunctionType

    nc = bacc.Bacc(
        "TRN2",
        target_bir_lowering=False,
        debug=False,
        enable_asserts=False,
        num_devices=1,
    )

    audio = nc.dram_tensor("audio", [B_CORE, 2, WIN], f32, kind="ExternalInput")
    out = nc.dram_tensor("out", [2, 128, 3], f32, kind="ExternalOutput")

    # DFT matrices
    t = np.arange(M)[:, None]
    f = np.arange(F)[None, :]
    ang = 2.0 * np.pi * t * f / M
    Fc = np.cos(ang).astype(np.float32)  # [M, F]
    Fsn = (-np.sin(ang)).astype(np.float32)
    j = np.arange(NLAGS)[None, :]
    alpha = np.where((np.arange(F) == 0) | (np.arange(F) == F - 1), 1.0, 2.0)[:, None]
    angj = 2.0 * np.pi * np.arange(F)[:, None] * j / M
    Gc = (alpha * np.cos(angj) / M).astype(np.float32)  # [F, 41]
    Gs3 = (-alpha * np.sin(angj) / M).astype(np.float32)
    Gs4 = (alpha * np.sin(angj) / M).astype(np.float32)

    consts = {
        "FcA": Fc[0:127, :],
        "FcC": Fc[127:254, :],
        "FsnA": Fsn[0:127, :],
        "FsnC": Fsn[127:254, :],
        "Gc": Gc,
        "Gs3": Gs3,
        "Gs4": Gs4,
        "ident": np.eye(128, dtype=np.float32),
        "theta_tab": np.tile(_theta_table(), (128, 1)),
        "iota_tab": np.tile(np.arange(NLAGS, dtype=np.float32), (128, 1)),
    }
    drams = {k: nc.inline_tensor(v, name=k) for k, v in consts.items()}

    W0 = 20 + WIN + (C * (K - 1) + M - (20 + WIN))  # a0buf width = 4962
    W1 = C * K  # a1buf width = 4922

    with tile.TileContext(nc) as tc:
        with (
            tc.tile_pool(name="consts", bufs=1) as cpool,
            tc.tile_pool(name="audio", bufs=1) as apool,
            tc.tile_pool(name="uvT", bufs=3) as upool,
            tc.tile_pool(name="prod", bufs=3) as ppool,
            tc.tile_pool(name="small", bufs=2) as mpool,
            tc.tile_pool(name="tp", bufs=2, space="PSUM") as tpp,
            tc.tile_pool(name="spec", bufs=2, space="PSUM") as spp,
            tc.tile_pool(name="cacc", bufs=1, space="PSUM") as cap,
            tc.tile_pool(name="ctp", bufs=1, space="PSUM") as ctp,
        ):
            ct_ = {}
            F32R_CONSTS = {"FcA", "FcC", "FsnA", "FsnC", "Gc", "Gs3", "Gs4"}
            for k, v in consts.items():
                dt_k = f32r if k in F32R_CONSTS else f32
                ct_[k] = cpool.tile(list(v.shape), dt_k, tag=k, name=k)
                if k in F32R_CONSTS:
                    nc.gpsimd.dma_start(out=ct_[k][:], in_=drams[k].ap())
                else:
                    nc.sync.dma_start(out=ct_[k][:], in_=drams[k].ap())

            a0b, a1b = [], []
            for h in range(2):
                b0 = h * 128
                a0 = apool.tile([128, W0], f32, tag=f"a0_{h}")
                a1 = apool.tile([128, W1], f32, tag=f"a1_{h}")
                nc.vector.memset(a0[:, 0:20], 0.0)
                nc.vector.memset(a0[:, 20 + WIN : W0], 0.0)
                nc.vector.memset(a1[:, WIN:W1], 0.0)
                nc.sync.dma_start(out=a0[:, 20 : 20 + WIN], in_=audio[b0 : b0 + 128, 0, :])
                nc.sync.dma_start(out=a1[:, 0:WIN], in_=audio[b0 : b0 + 128, 1, :])
                a0b.append(a0)
                a1b.append(a1)

            corrps = cap.tile([NLAGS, 256], f32, tag="corrps")

            for k in range(K):
                base = C * k
                # --- transposes: split A rows 0:127, split B rows 127:ulen/vlen
                uvT = []
                for s, ulen, vlen in ((0, 127, 127), (1, 87, 127)):
                    tp = tpp.tile([128, 512], f32, tag="tp")
                    off = base + 127 * s
                    for h in range(2):
                        nc.tensor.transpose(
                            tp[0:ulen, 128 * h : 128 * h + 128],
                            a1b[h][:, off : off + ulen],
                            ct_["ident"][:],
                        )
                        nc.tensor.transpose(
                            tp[0:vlen, 256 + 128 * h : 384 + 128 * h],
                            a0b[h][:, off : off + vlen],
                            ct_["ident"][:],
                        )
                    uv = upool.tile([128, 512], f32r, tag="uvT")
                    nc.scalar.copy(uv[0:127, :], tp[0:127, :])
                    uvT.append(uv)

                # --- forward DFT matmuls (float32r)
                U = spp.tile([128, 512], f32, tag="U")
                V = spp.tile([128, 512], f32, tag="V")
                r = lambda ap: ap
                for dst, mA, mC in (
                    (U[:, 0:256], "FcA", "FcC"),
                    (U[:, 256:512], "FsnA", "FsnC"),
                ):
                    nc.tensor.matmul(
                        dst, r(ct_[mA][:]), r(uvT[0][0:127, 0:256]),
                        start=True, stop=False,
                    )
                    nc.tensor.matmul(
                        dst, r(ct_[mC][0:87, :]), r(uvT[1][0:87, 0:256]),
                        start=False, stop=True,
                    )
                for dst, mA, mC in (
                    (V[:, 0:256], "FcA", "FcC"),
                    (V[:, 256:512], "FsnA", "FsnC"),
                ):
                    nc.tensor.matmul(
                        dst, r(ct_[mA][:]), r(uvT[0][0:127, 256:512]),
                        start=True, stop=False,
                    )
                    nc.tensor.matmul(
                        dst, r(ct_[mC][:]), r(uvT[1][0:127, 256:512]),
                        start=False, stop=True,
                    )

                # --- cross-spectrum products (DVE): Sr=m1+m2, Si=m3-m4
                Vsb = ppool.tile([128, 512], f32, tag="Vsb")
                nc.scalar.copy(Vsb[:], V[:])
                p12 = ppool.tile([128, 512], f32r, tag="p12")
                p34 = ppool.tile([128, 512], f32r, tag="p34")
                nc.vector.tensor_mul(out=p12[:], in0=U[:], in1=Vsb[:])
                nc.vector.tensor_mul(out=p34[:, 0:256], in0=U[:, 0:256], in1=Vsb[:, 256:512])
                nc.vector.tensor_mul(out=p34[:, 256:512], in0=U[:, 256:512], in1=Vsb[:, 0:256])

                # --- inverse DFT accumulate (float32r)
                nc.tensor.matmul(
                    corrps[:], r(ct_["Gc"][:]), r(p12[:, 0:256]),
                    start=(k == 0), stop=False, skip_group_check=True,
                )
                nc.tensor.matmul(
                    corrps[:], r(ct_["Gc"][:]), r(p12[:, 256:512]),
                    start=False, stop=False, skip_group_check=True,
                )
                nc.tensor.matmul(
                    corrps[:], r(ct_["Gs3"][:]), r(p34[:, 0:256]),
                    start=False, stop=False, skip_group_check=True,
                )
                nc.tensor.matmul(
                    corrps[:], r(ct_["Gs4"][:]), r(p34[:, 256:512]),
                    start=False, stop=(k == K - 1), skip_group_check=True,
                )

            # --- tail
            corrSB = mpool.tile([NLAGS, 256], f32, tag="corrSB")
            nc.scalar.copy(corrSB[:], corrps[:])
            for h in range(2):
                ctt = ctp.tile([128, NLAGS], f32, tag="ct")
                nc.tensor.transpose(
                    ctt[:], corrSB[:, 128 * h : 128 * h + 128], ct_["ident"][0:NLAGS, 0:NLAGS]
                )
                cabs = mpool.tile([128, NLAGS], f32, tag="cabs")
                nc.scalar.activation(cabs[:], ctt[:], Act.Abs)
                m8 = mpool.tile([128, 8], f32, tag="m8")
                i8 = mpool.tile([128, 8], mybir.dt.uint32, tag="i8")
                nc.vector.max_with_indices(m8[:], i8[:], cabs[:])
                ot = mpool.tile([128, 3], f32, tag="ot")
                idxf = mpool.tile([128, 1], f32, tag="idxf")
                nc.vector.tensor_copy(out=idxf[:], in_=i8[:, 0:1])
                nc.vector.tensor_scalar_add(ot[:, 0:1], idxf[:], -float(MAX_DELAY))
                onehot = mpool.tile([128, NLAGS], f32, tag="onehot")
                nc.vector.tensor_scalar(
                    out=onehot[:], in0=ct_["iota_tab"][:], scalar1=idxf[:],
                    scalar2=None, op0=Alu.is_equal,
                )
                junk41 = mpool.tile([128, NLAGS], f32, tag="junk41")
                nc.vector.scalar_tensor_tensor(
                    out=junk41[:], in0=onehot[:], scalar=1.0, in1=ct_["theta_tab"][:],
                    op0=Alu.mult, op1=Alu.mult, accum_out=ot[:, 1:2],
                )
                denom = mpool.tile([128, 1], f32, tag="denom")
                nc.vector.tensor_scalar_add(denom[:], m8[:, 1:2], 1e-9)
                recip = mpool.tile([128, 1], f32, tag="recip")
                nc.vector.reciprocal(recip[:], denom[:])
                ratio = mpool.tile([128, 1], f32, tag="ratio")
                nc.vector.tensor_mul(out=ratio[:], in0=m8[:, 0:1], in1=recip[:])
                nc.scalar.activation(ot[:, 2:3], ratio[:], Act.Tanh, scale=0.5)
                nc.sync.dma_start(out=out[h, :, :], in_=ot[:])

    nc.compile()
    return nc


def _get_program():
    if "nc" not in _cache:
        _cache["nc"] = _build()
    return _cache["nc"]


def _run(audio_np, trace=False):
    from concourse.bass_utils import run_bass_kernel_spmd

    nc = _get_program()
    in_maps = [
        {"audio": np.ascontiguousarray(audio_np[i * B_CORE : (i + 1) * B_CORE])}
        for i in range(N_CORES)
    ]
    res = run_bass_kernel_spmd(nc, in_maps, core_ids=list(range(N_CORES)), trace=trace)
    packed = np.concatenate([r["out"].reshape(B_CORE, 3) for r in res.results], axis=0)
    lag = np.rint(packed[:, 0]).astype(np.int32)
    return (lag, packed[:, 1].astype(np.float32), packed[:, 2].astype(np.float32)), res


def kernel(audio):
    audio_np = np.ascontiguousarray(np.asarray(audio, dtype=np.float32))
    assert audio_np.shape == (BATCH, 2, WIN), audio_np.shape
    (lag, theta, conf), _ = _run(audio_np, trace=False)
    return lag, theta, conf


def kernel_profiled(audio):
    audio_np = np.ascontiguousarray(np.asarray(audio, dtype=np.float32))
    (lag, theta, conf), res = _run(audio_np, trace=True)
    return (lag, theta, conf), res

